# revision 7
# baseline (speedup 1.0000x reference)
"""Trainium2 Bass kernel for nn_ContinuousThoughtBlock (v2: sharded thought MLP).

Strategy: batch-parallel for h/gate/output (core r owns batch r), but the
thought-phase MLP is model-parallel over H across all 8 cores:

  context_b = mean_L(h_b)            computed on core b, AllGathered (tiny)
  every core then evolves ALL 64 (path, batch) columns through the 4
  residual-MLP steps, but only contracts its own H-slice (W1[:, r*512:],
  W2[r*512:, :]); the partial dense2 outputs are AllReduced (128KB bf16)
  per step.  PE efficiency per matmul rises from 8 to 64 streamed columns
  and per-core weight DMA drops 16MB -> 2MB.

  amps/collapse/bc are computed replicated for all 8 batches; a one-hot
  `bsel` input selects the core's own bc row (SPMD graphs must be
  identical across cores, so per-core choices ride in via inputs).

  gate = sigmoid(h @ Wg + bg) stays resident in SBUF (no DRAM spill);
  out = LN_D(h + gate * bc) written as bf16 (host converts to f32).

  A dummy AllGather fires at t=0 so the ~60us one-time collective-init
  overlaps the h/weight loads instead of the first real AllReduce.

Queues: sync = h, W1s/W2s, Wbc, AG/AR bounces; scalar = Wg/Wagg, hT
transposes, out writes; gpsimd = collectives (+ casting loads on the
nontrivial-vector paths only).
"""

import numpy as np

import concourse.bass as bass
import concourse.mybir as mybir
import concourse.tile as tile
from concourse import bacc
from concourse.bass_utils import run_bass_kernel_spmd
from concourse.masks import make_identity

# Problem constants (hardcoded per harness contract).
B, L, D, H = 8, 2048, 1024, 4096
NUM_PATHS = 8
NUM_STEPS = 4
PRUNE = 0.1
EPS = 1e-6
KD = D // 128     # 8  D-chunks
ML = L // 128     # 16 L-tiles
HSL = H // B      # 512 per-core H slice
KHS = HSL // 128  # 4  H-slice chunks
NC = 64           # thought columns = B batches x NUM_PATHS paths, c = b*8+p
INV_SQRT_D = 1.0 / float(np.sqrt(np.float32(D)))

F32 = mybir.dt.float32
BF16 = mybir.dt.bfloat16
AF = mybir.ActivationFunctionType
ALU = mybir.AluOpType
AX = mybir.AxisListType
RG = [list(range(B))]

WEIGHT_NAMES = [
    "input_norm_gamma", "input_norm_beta",
    "aggregator_weight", "aggregator_bias",
    "projector_norm_gamma", "projector_norm_beta",
    "projector_dense1_weight", "projector_dense1_bias",
    "projector_dense2_weight", "projector_dense2_bias",
    "broadcast_weight", "broadcast_bias",
    "gate_weight", "gate_bias",
    "output_norm_gamma", "output_norm_beta",
]


def _bc0(ap, n=128):
    """Broadcast a 1-D AP down n partitions via a stride-0 partition dim."""
    return bass.AP(tensor=ap.tensor, offset=ap.offset, ap=[[0, n]] + list(ap.ap))


def _rep0(ap, n, pos=1):
    """Insert a stride-0 free dim of extent n at position pos."""
    new = list(ap.ap)
    new.insert(pos, [0, n])
    return bass.AP(tensor=ap.tensor, offset=ap.offset, ap=new)


def build_graph(triv, debug=False):
    nc = bacc.Bacc("TRN2", target_bir_lowering=False, debug=False,
                   enable_asserts=True, num_devices=B)

    h_ext = nc.declare_dram_parameter("hidden_states", [L, D], BF16, isOutput=False)
    w_ext = {}
    for n in ("aggregator_weight", "broadcast_weight", "gate_weight"):
        w_ext[n] = nc.declare_dram_parameter(n, [D, D], BF16, isOutput=False)
    w_ext["projector_dense1_weight"] = nc.declare_dram_parameter(
        "projector_dense1_weight", [D, HSL], BF16, isOutput=False)
    w_ext["projector_dense2_weight"] = nc.declare_dram_parameter(
        "projector_dense2_weight", [HSL, D], BF16, isOutput=False)
    for n in ("input_norm_gamma", "input_norm_beta", "aggregator_bias",
              "projector_norm_gamma", "projector_norm_beta",
              "projector_dense1_bias", "projector_dense2_bias",
              "broadcast_bias", "gate_bias",
              "output_norm_gamma", "output_norm_beta"):
        shape = [HSL] if n == "projector_dense1_bias" else [D]
        w_ext[n] = nc.declare_dram_parameter(n, shape, F32, isOutput=False)
    w_ext["bsel"] = nc.declare_dram_parameter("bsel", [B, 1], F32, isOutput=False)
    out_ext = nc.declare_dram_parameter("out", [L, D], BF16, isOutput=True)
    dbg = {}
    if debug:
        for nm, shape in (("d_ctxrows", [B, D]), ("d_ctxn", [B, D]),
                          ("d_th0", [128, KD * B]), ("d_tT", [128, KD * NC]),
                          ("d_amps", [1, NC]), ("d_y0", [128, KD * NC]),
                          ("d_bc", [B, D]), ("d_gate0", [128, D])):
            dbg[nm] = nc.declare_dram_parameter(nm, shape, F32, isOutput=True)

    with tile.TileContext(nc) as tc:
        _build_body(nc, tc, h_ext, w_ext, out_ext, triv, dbg)
    nc.compile()
    return nc


def _dmajor(nc, pool, ps_pool, ident_bf, dram_ap, n, name):
    """DMA a [n*128] DRAM vector into a [128, n] d-major SBUF tile
    (tile[p, k] = v[k*128 + p]) via a bf16 [n,128] load + PE transpose."""
    rowk = pool.tile([n, 128], BF16, name="dmaj_rowk")
    nc.gpsimd.dma_start(out=rowk[:], in_=dram_ap.rearrange("(k p) -> k p", p=128))
    ps = ps_pool.tile([128, n], BF16, name="dmaj_ps")
    nc.tensor.transpose(ps[:], rowk[:], ident_bf[0:n, 0:n])
    t = pool.tile([128, n], F32, name=name)
    nc.scalar.copy(t[:], ps[:])
    return t


def _build_body(nc, tc, h_ext, w, out_ext, triv, dbg=None):
    dbg = dbg or {}
    import contextlib
    ctx = contextlib.ExitStack()
    with ctx:
        # ---------------- pools ----------------
        singles = ctx.enter_context(tc.tile_pool(name="singles", bufs=1))
        smalls = ctx.enter_context(tc.tile_pool(name="smalls", bufs=1))
        tstate = ctx.enter_context(tc.tile_pool(name="tstate", bufs=3))
        hTm_pool = ctx.enter_context(tc.tile_pool(name="hTm", bufs=2))
        rows = ctx.enter_context(tc.tile_pool(name="rows", bufs=1))
        wpool = tc.alloc_tile_pool(name="wpool", bufs=1)
        dram = ctx.enter_context(tc.tile_pool(name="dram", bufs=1, space="DRAM"))

        ps_small = ctx.enter_context(tc.tile_pool(name="ps_small", bufs=2, space="PSUM"))
        ps_tr = ctx.enter_context(tc.tile_pool(name="ps_tr", bufs=1, space="PSUM"))
        ps_gate = ctx.enter_context(tc.tile_pool(name="ps_gate", bufs=3, space="PSUM"))
        ps_th = ctx.enter_context(tc.tile_pool(name="ps_th", bufs=2, space="PSUM"))

        # ---------------- collective bounce buffers ----------------
        agd_in = dram.tile([1, 8], F32)
        agd_out = dram.tile([B, 8], F32)
        ag_in = dram.tile([KD, 128], F32)
        ag_out = dram.tile([B * KD, 128], F32)
        y_in = [dram.tile([128, 512], BF16, name=f"y_in{s}") for s in range(NUM_STEPS)]
        y_out = [dram.tile([128, 512], BF16, name=f"y_out{s}") for s in range(NUM_STEPS)]

        # ---------------- constants ----------------
        ident_bf = singles.tile([128, 128], BF16)
        make_identity(nc, ident_bf[:])
        ident_f32 = singles.tile([128, 128], F32)
        nc.vector.tensor_copy(ident_f32[:], ident_bf[:])
        ones_bf = singles.tile([128, 1], BF16)
        nc.vector.memset(ones_bf[:], 1.0)
        ones_f32 = singles.tile([128, 1], F32)
        nc.vector.memset(ones_f32[:], 1.0)
        ones_row = singles.tile([1, 128], F32)
        nc.vector.memset(ones_row[:], 1.0)
        ones_row_bf = singles.tile([1, 128], BF16)
        nc.vector.memset(ones_row_bf[:], 1.0)
        eps1 = singles.tile([1, 1], F32)
        nc.vector.memset(eps1[:], EPS)
        eps_col = singles.tile([128, 1], F32)
        nc.vector.memset(eps_col[:], EPS)

        # dummy collective to absorb the one-time init under the loads
        zdum = smalls.tile([1, 8], F32, name="zdum")
        nc.vector.memset(zdum[:], 0.0)
        nc.sync.dma_start(out=agd_in[:], in_=zdum[:])
        nc.gpsimd.collective_compute(
            "AllGather", ALU.bypass, replica_groups=RG,
            ins=[agd_in.opt()], outs=[agd_out.opt()])

        # resident (bf16) tensors
        h_bf = singles.tile([128, ML, D], BF16)      # 32KB/part
        gate_sb = singles.tile([128, ML, D], BF16)   # 32KB/part
        wg_bf = wpool.tile([128, KD, D], BF16)       # 16KB/part
        wagg_bf = wpool.tile([128, KD, D], BF16)     # 16KB/part
        wbc_bf = wpool.tile([128, KD, D], BF16)      # 16KB/part
        w1s_bf = wpool.tile([128, KD, HSL], BF16)    # 8KB/part
        w2s_bf = wpool.tile([128, KHS, D], BF16)     # 8KB/part

        bsel_sb = smalls.tile([B, 1], F32, name="bsel_sb")
        nc.sync.dma_start(out=bsel_sb[:], in_=w["bsel"].ap())

        # d-major vectors (only when nontrivial)
        gammaT_pr = betaT_pr = None
        if not triv["projector_norm"]:
            gammaT_pr = _dmajor(nc, singles, ps_tr, ident_bf,
                                w["projector_norm_gamma"].ap(), KD, "g_pr")
            betaT_pr = _dmajor(nc, singles, ps_tr, ident_bf,
                               w["projector_norm_beta"].ap(), KD, "b_pr")
        baggT = None
        if not triv["aggregator_bias"]:
            baggT = _dmajor(nc, singles, ps_tr, ident_bf,
                            w["aggregator_bias"].ap(), KD, "bagg")
        b1T = None
        if not triv["projector_dense1_bias"]:
            b1T = _dmajor(nc, singles, ps_tr, ident_bf,
                          w["projector_dense1_bias"].ap(), KHS, "b1")
        b2T_rep = None
        if not triv["projector_dense2_bias"]:
            b2T = _dmajor(nc, singles, ps_tr, ident_bf,
                          w["projector_dense2_bias"].ap(), KD, "b2")
            b2T_rep = _rep0(b2T[:], NC, pos=2)  # [128, KD, NC] view
        gbias_row = None
        if not triv["gate_bias"]:
            gbias_row = rows.tile([1, D], BF16, name="gbrow")
            nc.gpsimd.dma_start(out=gbias_row[:],
                                in_=w["gate_bias"].ap().rearrange("(a d) -> a d", a=1))
        gin_row = bin_row = None
        if not triv["input_norm"]:
            gin_row = rows.tile([1, D], F32, name="ginrow")
            nc.sync.dma_start(out=gin_row[:],
                              in_=w["input_norm_gamma"].ap().rearrange("(a d) -> a d", a=1))
            bin_row = rows.tile([1, D], F32, name="binrow")
            nc.sync.dma_start(out=bin_row[:],
                              in_=w["input_norm_beta"].ap().rearrange("(a d) -> a d", a=1))

        # ---------------- loads ----------------
        h_src = h_ext.ap().rearrange("(m t p) d -> p m t d", p=128, t=2)
        for m2 in range(ML // 2):
            nc.sync.dma_start(out=h_bf[:, 2 * m2:2 * m2 + 2, :], in_=h_src[:, m2])
        wg_src = w["gate_weight"].ap().rearrange("(k t p) d -> p k t d", p=128, t=2)
        for k2 in range(KD // 2):
            nc.scalar.dma_start(out=wg_bf[:, 2 * k2:2 * k2 + 2, :], in_=wg_src[:, k2])
        wagg_src = w["aggregator_weight"].ap().rearrange("(k t p) d -> p k t d",
                                                         p=128, t=2)
        for k2 in range(KD // 2):
            nc.scalar.dma_start(out=wagg_bf[:, 2 * k2:2 * k2 + 2, :],
                                in_=wagg_src[:, k2])
        w1_src = w["projector_dense1_weight"].ap().rearrange("(k p) h -> p k h", p=128)
        nc.sync.dma_start(out=w1s_bf[:], in_=w1_src)
        w2_src = w["projector_dense2_weight"].ap().rearrange("(k p) d -> p k d", p=128)
        nc.sync.dma_start(out=w2s_bf[:], in_=w2_src)
        wbc_src = w["broadcast_weight"].ap().rearrange("(k t p) d -> p k t d",
                                                       p=128, t=2)
        for k2 in range(KD // 2):
            nc.sync.dma_start(out=wbc_bf[:, 2 * k2:2 * k2 + 2, :], in_=wbc_src[:, k2])

        # ---------------- gate tiles (emitted in slices to fill PE) ---------
        def emit_gate_tiles(ms):
            for m in ms:
                hTm = hTm_pool.tile([128, KD, 128], BF16, name="hTm")
                nc.scalar.dma_start_transpose(out=hTm[:], in_=h_bf[:, m, :])
                for n in range(2):
                    g_ps = ps_gate.tile([128, 512], F32, name="g_ps", tag="gps")
                    for k in range(KD):
                        nc.tensor.matmul(g_ps[:], hTm[:, k, :],
                                         wg_bf[:, k, n * 512:(n + 1) * 512],
                                         start=(k == 0),
                                         stop=(k == KD - 1 and gbias_row is None))
                    if gbias_row is not None:
                        nc.tensor.matmul(g_ps[:], ones_row_bf[:],
                                         gbias_row[0:1, n * 512:(n + 1) * 512],
                                         start=False, stop=True)
                    nc.scalar.activation(gate_sb[:, m, n * 512:(n + 1) * 512],
                                         g_ps[:], AF.Sigmoid)

        # ---------------- context (own batch), d-major ----------------
        ctx_ps = ps_small.tile([128, KD], F32, name="ctx_ps", tag="sm")
        emit_gate_tiles(range(0, 2))
        for k in range(KD):
            for m in range(ML):
                nc.tensor.matmul(ctx_ps[:, k:k + 1],
                                 h_bf[:, m, k * 128:(k + 1) * 128],
                                 ones_bf[:],
                                 start=(m == 0), stop=(m == ML - 1))
        ctxT = singles.tile([128, KD], F32)
        nc.scalar.mul(ctxT[:], ctx_ps[:], 1.0 / L)

        # transpose to a [KD, 128] row block and AllGather all batches' ctx
        ctxrow_ps = ps_tr.tile([KD, 128], F32, name="ctxrow_ps", tag="tr")
        nc.tensor.matmul(ctxrow_ps[:], ctxT[:], ident_f32[:], start=True, stop=True)
        ctxrow = smalls.tile([KD, 128], F32, name="ctxrow")
        nc.scalar.copy(ctxrow[:], ctxrow_ps[:])
        nc.sync.dma_start(out=ag_in[:], in_=ctxrow[:])
        nc.gpsimd.collective_compute(
            "AllGather", ALU.bypass, replica_groups=RG,
            ins=[ag_in.opt()], outs=[ag_out.opt()])
        ctx_rows = smalls.tile([B, D], F32, name="ctx_rows")
        nc.sync.dma_start(out=ctx_rows[:],
                          in_=ag_out[:].rearrange("(r k) p -> r (k p)", r=B))
        if "d_ctxrows" in dbg:
            nc.sync.dma_start(out=dbg["d_ctxrows"].ap(), in_=ctx_rows[:])

        emit_gate_tiles(range(2, 5))

        # ---------------- input LN on ctx rows [B, D] ----------------
        rs2 = smalls.tile([B, 2], F32, name="rs2")   # [sum, sumsq]
        nc.vector.tensor_reduce(rs2[:, 0:1], ctx_rows[:], axis=AX.X, op=ALU.add)
        csq = smalls.tile([B, D], F32, name="csq")
        nc.scalar.activation(csq[:], ctx_rows[:], AF.Square, accum_out=rs2[:, 1:2])
        cmv = smalls.tile([B, 2], F32, name="cmv")   # [mean, E[x^2]]
        nc.vector.tensor_scalar(cmv[:], rs2[:], 1.0 / D, None, op0=ALU.mult)
        cvar = smalls.tile([B, 1], F32, name="cvar")
        nc.vector.tensor_tensor(out=cvar[:], in0=cmv[:, 0:1], in1=cmv[:, 0:1],
                                op=ALU.mult)
        nc.vector.tensor_sub(cvar[:], cmv[:, 1:2], cvar[:])
        nc.scalar.activation(cvar[:], cvar[:], AF.Sqrt, bias=eps_col[0:B, :])
        crstd = smalls.tile([B, 1], F32, name="crstd")
        nc.vector.reciprocal(crstd[:], cvar[:])
        ctx_n = smalls.tile([B, D], F32, name="ctx_n")
        nc.vector.tensor_scalar(ctx_n[:], ctx_rows[:], cmv[:, 0:1], crstd[:, 0:1],
                                op0=ALU.subtract, op1=ALU.mult)
        if gin_row is not None:
            nc.vector.tensor_mul(ctx_n[:], ctx_n[:], _bc0(gin_row[0, :], B))
            nc.vector.tensor_add(ctx_n[:], ctx_n[:], _bc0(bin_row[0, :], B))
        if "d_ctxn" in dbg:
            nc.sync.dma_start(out=dbg["d_ctxn"].ap(), in_=ctx_n[:])
        ctxn_bf = smalls.tile([B, D], BF16, name="ctxn_bf")
        nc.vector.tensor_copy(ctxn_bf[:], ctx_n[:])
        ctxr_bf = smalls.tile([B, D], BF16, name="ctxr_bf")
        nc.vector.tensor_copy(ctxr_bf[:], ctx_rows[:])

        # transpose ctx_n and ctx rows to d-major [128, KD, B]
        ctx_nT = smalls.tile([128, KD, B], BF16, name="ctx_nT")
        ctxA_bf = smalls.tile([128, KD, B], BF16, name="ctxA_bf")
        for k in range(KD):
            tp = ps_tr.tile([128, B], F32, name="tp", tag="tr")
            nc.tensor.matmul(tp[:], ctxn_bf[:, k * 128:(k + 1) * 128],
                             ident_bf[0:B, 0:B], start=True, stop=True)
            nc.scalar.copy(ctx_nT[:, k, :], tp[:])
            tp2 = ps_tr.tile([128, B], F32, name="tp2", tag="tr")
            nc.tensor.matmul(tp2[:], ctxr_bf[:, k * 128:(k + 1) * 128],
                             ident_bf[0:B, 0:B], start=True, stop=True)
            nc.scalar.copy(ctxA_bf[:, k, :], tp2[:])

        # ---------------- thought0 = ctx_n @ Wagg  [128, KD, B] ----------------
        th0_ps = ps_th.tile([128, KD * B], F32, name="th0_ps", tag="th")
        for dm in range(KD):
            for k in range(KD):
                nc.tensor.matmul(th0_ps[:, dm * B:(dm + 1) * B],
                                 wagg_bf[:, k, dm * 128:(dm + 1) * 128],
                                 ctx_nT[:, k, :],
                                 start=(k == 0), stop=(k == KD - 1))
        th0 = smalls.tile([128, KD, B], F32, name="th0")
        if baggT is not None:
            nc.vector.tensor_tensor(out=th0[:],
                                    in0=th0_ps[:].rearrange("a (k b) -> a k b", k=KD),
                                    in1=_rep0(baggT[:], B, pos=2), op=ALU.add)
        else:
            nc.scalar.copy(th0[:].rearrange("a k b -> a (k b)"), th0_ps[:])
        if "d_th0" in dbg:
            nc.sync.dma_start(out=dbg["d_th0"].ap(),
                              in_=th0[:].rearrange("a k b -> a (k b)"))

        # seed 64 columns: tT[:, k, b*8+p] = th0[:, k, b] * (1 + 0.02 p)
        tT = tstate.tile([128, KD, NC], F32, name="tT")
        tT_bp = tT[:].rearrange("a k (b p) -> a k b p", p=NUM_PATHS)
        for p in range(NUM_PATHS):
            nc.scalar.mul(tT_bp[:, :, :, p], th0[:], 1.0 + 0.02 * p)

        emit_gate_tiles(range(5, 7))

        # ---------------- thought steps ----------------
        _gate_slices = {0: range(7, 10), 1: range(10, 13), 2: range(13, 15),
                        3: range(15, 16)}
        ampsF = None
        for step in range(NUM_STEPS):
            last = step == NUM_STEPS - 1
            # LN stats over D for all 64 columns (ones-matmul partition sums)
            sq = smalls.tile([128, KD, NC], F32, name="sq")
            nc.vector.tensor_mul(sq[:], tT[:], tT[:])
            st_a = ps_small.tile([1, 512], F32, name="st_a", tag="sm")
            st_b = ps_small.tile([1, 512], F32, name="st_b", tag="sm")
            nc.tensor.matmul(st_a[:], ones_f32[:],
                             tT[:].rearrange("a k q -> a q k"), start=True, stop=True)
            nc.tensor.matmul(st_b[:], ones_f32[:],
                             sq[:].rearrange("a k q -> a q k"), start=True, stop=True)
            # ms = [mean(64) | rstd(64)] on one partition
            ms = smalls.tile([1, 2 * NC], F32, name="ms")
            sums = smalls.tile([1, 2 * NC], F32, name="sums")
            nc.vector.tensor_reduce(sums[0:1, 0:NC],
                                    st_a[:].rearrange("a (q k) -> a q k", k=KD),
                                    axis=AX.X, op=ALU.add)
            nc.vector.tensor_reduce(sums[0:1, NC:2 * NC],
                                    st_b[:].rearrange("a (q k) -> a q k", k=KD),
                                    axis=AX.X, op=ALU.add)
            nc.scalar.mul(ms[:], sums[:], 1.0 / D)
            m2 = smalls.tile([1, NC], F32, name="m2")
            nc.vector.tensor_mul(m2[0:1, :], ms[0:1, 0:NC], ms[0:1, 0:NC])
            var = smalls.tile([1, NC], F32, name="var")
            nc.vector.tensor_sub(var[0:1, :], ms[0:1, NC:2 * NC], m2[0:1, :])
            nc.scalar.activation(var[0:1, :], var[0:1, :], AF.Sqrt, bias=eps1[0:1, :])
            nc.vector.reciprocal(ms[0:1, NC:2 * NC], var[0:1, :])
            # broadcast [1, 2*NC] -> [128, 2*NC] via ones-row matmul
            mr_ps = ps_small.tile([128, 2 * NC], F32, name="mr_ps", tag="sm")
            nc.tensor.matmul(mr_ps[:], ones_row[:], ms[:], start=True, stop=True)

            # normalize all (k, q) at once with stride-0 broadcasts
            tn_bf = smalls.tile([128, KD, NC], BF16, name="tn_bf")
            tc_f = smalls.tile([128, KD, NC], F32, name="tc_f")
            nc.vector.tensor_tensor(out=tc_f[:], in0=tT[:],
                                    in1=_rep0(mr_ps[:, 0:NC], KD),
                                    op=ALU.subtract)
            if gammaT_pr is not None:
                nc.vector.tensor_tensor(out=tc_f[:], in0=tc_f[:],
                                        in1=_rep0(mr_ps[:, NC:2 * NC], KD),
                                        op=ALU.mult)
                nc.vector.tensor_tensor(out=tc_f[:], in0=tc_f[:],
                                        in1=_rep0(gammaT_pr[:], NC, pos=2),
                                        op=ALU.mult)
                nc.vector.tensor_tensor(out=tn_bf[:], in0=tc_f[:],
                                        in1=_rep0(betaT_pr[:], NC, pos=2),
                                        op=ALU.add)
            else:
                nc.vector.tensor_tensor(out=tn_bf[:], in0=tc_f[:],
                                        in1=_rep0(mr_ps[:, NC:2 * NC], KD),
                                        op=ALU.mult)

            # dense1 on my H slice: x1 [128, KHS, NC]
            x1_ps = ps_th.tile([128, KHS * NC], F32, name="x1_ps", tag="th")
            for hs in range(KHS):
                for k in range(KD):
                    nc.tensor.matmul(x1_ps[:, hs * NC:(hs + 1) * NC],
                                     w1s_bf[:, k, hs * 128:(hs + 1) * 128],
                                     tn_bf[:, k, :],
                                     start=(k == 0), stop=(k == KD - 1))
            # tanh-gelu (matches jax.nn.gelu approximate=True)
            xs = smalls.tile([128, KHS * NC], F32, name="gelu_x")
            if b1T is not None:
                for hs in range(KHS):
                    nc.scalar.activation(xs[:, hs * NC:(hs + 1) * NC],
                                         x1_ps[:, hs * NC:(hs + 1) * NC],
                                         AF.Identity, bias=b1T[:, hs:hs + 1])
            else:
                nc.scalar.copy(xs[:], x1_ps[:])
            u = smalls.tile([128, KHS * NC], F32, name="gelu_u")
            nc.vector.tensor_mul(u[:], xs[:], xs[:])
            nc.vector.tensor_mul(u[:], u[:], xs[:])
            nc.vector.scalar_tensor_tensor(u[:], u[:], 0.044715, xs[:],
                                           op0=ALU.mult, op1=ALU.add)
            nc.scalar.activation(u[:], u[:], AF.Tanh, scale=0.7978845608028654)
            nc.vector.scalar_tensor_tensor(u[:], u[:], 1.0, xs[:],
                                           op0=ALU.add, op1=ALU.mult)
            x1_bf = smalls.tile([128, KHS, NC], BF16, name="x1_bf")
            nc.scalar.mul(x1_bf[:].rearrange("a b c -> a (b c)"), u[:], 0.5)

            # dense2 partial: y [128, KD*NC]
            y_ps = ps_th.tile([128, KD * NC], F32, name="y_ps", tag="th")
            for dm in range(KD):
                for hk in range(KHS):
                    nc.tensor.matmul(y_ps[:, dm * NC:(dm + 1) * NC],
                                     w2s_bf[:, hk, dm * 128:(dm + 1) * 128],
                                     x1_bf[:, hk, :],
                                     start=(hk == 0), stop=(hk == KHS - 1))
            y_sb = smalls.tile([128, KD * NC], BF16, name="y_sb")
            nc.scalar.copy(y_sb[:], y_ps[:])
            nc.sync.dma_start(out=y_in[step][:], in_=y_sb[:])
            nc.gpsimd.collective_compute(
                "AllReduce", ALU.add, replica_groups=RG,
                ins=[y_in[step].opt()], outs=[y_out[step].opt()])
            if step in _gate_slices:
                emit_gate_tiles(_gate_slices[step])
            y_rd = smalls.tile([128, KD * NC], BF16, name="y_rd")
            nc.sync.dma_start(out=y_rd[:], in_=y_out[step][:])
            if step == 0 and "d_y0" in dbg:
                y_dbg = smalls.tile([128, KD * NC], F32, name="y_dbg")
                nc.vector.tensor_copy(y_dbg[:], y_rd[:])
                nc.sync.dma_start(out=dbg["d_y0"].ap(), in_=y_dbg[:])

            tT_new = tstate.tile([128, KD, NC], F32, name="tT_new")
            yv = y_rd[:].rearrange("a (k q) -> a k q", k=KD)
            if b2T_rep is not None:
                nc.vector.tensor_add(tT_new[:], yv, b2T_rep)
                nc.vector.tensor_add(tT_new[:], tT_new[:], tT[:])
            else:
                nc.vector.tensor_add(tT_new[:], yv, tT[:])
            tT = tT_new

            if last:
                tT_bf = smalls.tile([128, KD, NC], BF16, name="tT_bf")
                nc.vector.tensor_copy(tT_bf[:], tT[:])
                # scores for all 64 columns: per batch b, dot with ctx_b
                sc_ps = ps_small.tile([1, NC], F32, name="sc_ps", tag="sm")
                for b in range(B):
                    for k in range(KD):
                        nc.tensor.matmul(sc_ps[0:1, b * 8:(b + 1) * 8],
                                         ctxA_bf[:, k, b:b + 1],
                                         tT_bf[:, k, b * 8:(b + 1) * 8],
                                         start=(k == 0), stop=(k == KD - 1))
                sc = smalls.tile([1, NC], F32, name="sc")
                nc.scalar.mul(sc[:], sc_ps[:], INV_SQRT_D)
                scv = sc[:].rearrange("a (b p) -> a b p", p=NUM_PATHS)
                negmax = smalls.tile([1, B], F32, name="negmax")
                nc.vector.tensor_reduce(negmax[:], scv, axis=AX.X, op=ALU.max,
                                        negate=True)
                exv = smalls.tile([1, NC], F32, name="exv")
                nc.vector.tensor_tensor(
                    out=exv[:].rearrange("a (b p) -> a b p", p=NUM_PATHS),
                    in0=scv, in1=_rep0(negmax[:], NUM_PATHS, pos=2), op=ALU.add)
                nc.scalar.activation(exv[:], exv[:], AF.Exp)
                esum = smalls.tile([1, B], F32, name="esum")
                nc.vector.tensor_reduce(
                    esum[:], exv[:].rearrange("a (b p) -> a b p", p=NUM_PATHS),
                    axis=AX.X, op=ALU.add)
                rsum = smalls.tile([1, B], F32, name="rsum")
                nc.vector.reciprocal(rsum[:], esum[:])
                amps0 = smalls.tile([1, NC], F32, name="amps0")
                nc.vector.tensor_tensor(
                    out=amps0[:].rearrange("a (b p) -> a b p", p=NUM_PATHS),
                    in0=exv[:].rearrange("a (b p) -> a b p", p=NUM_PATHS),
                    in1=_rep0(rsum[:], NUM_PATHS, pos=2), op=ALU.mult)
                mask = smalls.tile([1, NC], F32, name="mask")
                nc.vector.tensor_scalar(mask[:], amps0[:], PRUNE, None, op0=ALU.is_ge)
                pruned = smalls.tile([1, NC], F32, name="pruned")
                nc.vector.tensor_mul(pruned[:], amps0[:], mask[:])
                psum_s = smalls.tile([1, B], F32, name="psum_s")
                nc.vector.tensor_reduce(
                    psum_s[:], pruned[:].rearrange("a (b p) -> a b p", p=NUM_PATHS),
                    axis=AX.X, op=ALU.add)
                nc.vector.tensor_scalar(psum_s[:], psum_s[:], EPS, None, op0=ALU.add)
                rr = smalls.tile([1, B], F32, name="rr")
                nc.vector.reciprocal(rr[:], psum_s[:])
                ampsF = smalls.tile([1, NC], F32, name="ampsF")
                nc.vector.tensor_tensor(
                    out=ampsF[:].rearrange("a (b p) -> a b p", p=NUM_PATHS),
                    in0=pruned[:].rearrange("a (b p) -> a b p", p=NUM_PATHS),
                    in1=_rep0(rr[:], NUM_PATHS, pos=2), op=ALU.mult)

        if "d_tT" in dbg:
            nc.sync.dma_start(out=dbg["d_tT"].ap(),
                              in_=tT[:].rearrange("a k q -> a (k q)"))
        if "d_amps" in dbg:
            nc.sync.dma_start(out=dbg["d_amps"].ap(), in_=ampsF[:])

        # ---------------- collapse + bc (all batches, then select own) -------
        ab_ps = ps_small.tile([128, NC], F32, name="ab_ps", tag="sm")
        nc.tensor.matmul(ab_ps[:], ones_row[0:1, :], ampsF[:], start=True, stop=True)
        amps_sb = smalls.tile([128, NC], F32, name="amps_sb")
        nc.scalar.copy(amps_sb[:], ab_ps[:])
        prod = smalls.tile([128, KD, NC], F32, name="prod")
        nc.vector.tensor_tensor(out=prod[:], in0=tT[:], in1=_rep0(amps_sb[:], KD),
                                op=ALU.mult)
        finalT = smalls.tile([128, KD, B], F32, name="finalT")
        nc.vector.tensor_reduce(
            finalT[:], prod[:].rearrange("a k (b p) -> a k b p", p=NUM_PATHS),
            axis=AX.X, op=ALU.add)
        finalT_bf = smalls.tile([128, KD, B], BF16, name="finalT_bf")
        nc.vector.tensor_copy(finalT_bf[:], finalT[:])

        # bc rows for ALL batches: [B, D] = finalT^T @ Wbc
        bc_all = smalls.tile([B, D], F32, name="bc_all")
        for n in range(2):
            bc_ps = ps_gate.tile([B, 512], F32, name="bc_ps", tag="gps")
            for k in range(KD):
                nc.tensor.matmul(bc_ps[:],
                                 finalT_bf[:, k, :],
                                 wbc_bf[:, k, n * 512:(n + 1) * 512],
                                 start=(k == 0), stop=(k == KD - 1))
            nc.scalar.copy(bc_all[:, n * 512:(n + 1) * 512], bc_ps[:])
        if not triv["broadcast_bias"]:
            bb_row = rows.tile([1, D], F32, name="bbrow")
            nc.sync.dma_start(out=bb_row[:],
                              in_=w["broadcast_bias"].ap().rearrange("(a d) -> a d", a=1))
            nc.vector.tensor_add(bc_all[:], bc_all[:], _bc0(bb_row[0, :], B))
        if "d_bc" in dbg:
            nc.sync.dma_start(out=dbg["d_bc"].ap(), in_=bc_all[:])

        # select own row via one-hot matmul, broadcast down 128 partitions
        bc_bf = singles.tile([128, D], BF16)
        for n in range(2):
            sel_ps = ps_gate.tile([1, 512], F32, name="sel_ps", tag="gps")
            nc.tensor.matmul(sel_ps[:], bsel_sb[:], bc_all[:, n * 512:(n + 1) * 512],
                             start=True, stop=True)
            sel_row = rows.tile([1, 512], F32, name="sel_row")
            nc.scalar.copy(sel_row[:], sel_ps[:])
            bcb_ps = ps_gate.tile([128, 512], F32, name="bcb_ps", tag="gps")
            nc.tensor.matmul(bcb_ps[:], ones_row[0:1, :], sel_row[:],
                             start=True, stop=True)
            nc.scalar.copy(bc_bf[:, n * 512:(n + 1) * 512], bcb_ps[:])

        # release weights; final-phase pools reuse the space
        wpool.release()
        fin = ctx.enter_context(tc.tile_pool(name="fin", bufs=6))
        gamma_out_b = beta_out_b = None
        if not triv["output_norm"]:
            fin1 = ctx.enter_context(tc.tile_pool(name="fin1", bufs=1))
            gamma_out_b = fin1.tile([128, D], F32)
            nc.sync.dma_start(out=gamma_out_b[:], in_=_bc0(w["output_norm_gamma"].ap()))
            beta_out_b = fin1.tile([128, D], F32)
            nc.sync.dma_start(out=beta_out_b[:], in_=_bc0(w["output_norm_beta"].ap()))

        # ---------------- final LN + output (bf16) ----------------
        if "d_gate0" in dbg:
            g_dbg = smalls.tile([128, D], F32, name="g_dbg")
            nc.vector.tensor_copy(g_dbg[:], gate_sb[:, 0, :])
            nc.sync.dma_start(out=dbg["d_gate0"].ap(), in_=g_dbg[:])
        for m in range(ML):
            p1 = fin.tile([128, D], BF16, name="p1")
            nc.vector.tensor_mul(p1[:], gate_sb[:, m, :], bc_bf[:])
            pre = fin.tile([128, D], BF16, name="pre")
            rs = fin.tile([128, 2], F32, name="rs")   # [rowsum, rowsumsq]
            nc.vector.scalar_tensor_tensor(pre[:], p1[:], 1.0, h_bf[:, m, :],
                                           op0=ALU.mult, op1=ALU.add,
                                           accum_out=rs[:, 0:1])
            sqs = fin.tile([128, D], BF16, name="sqs")
            nc.scalar.activation(sqs[:], pre[:], AF.Square, accum_out=rs[:, 1:2])
            mv = fin.tile([128, 2], F32, name="mv")   # [mean, E[x^2]]
            nc.vector.tensor_scalar(mv[:], rs[:], 1.0 / D, None, op0=ALU.mult)
            var = fin.tile([128, 1], F32, name="var_f")
            nc.vector.tensor_tensor(out=var[:], in0=mv[:, 0:1], in1=mv[:, 0:1],
                                    op=ALU.mult)
            nc.vector.tensor_sub(var[:], mv[:, 1:2], var[:])
            sd = fin.tile([128, 1], F32, name="sd")
            nc.scalar.activation(sd[:], var[:], AF.Sqrt, bias=eps_col[:])
            rstd = fin.tile([128, 1], F32, name="rstd")
            nc.vector.reciprocal(rstd[:], sd[:])
            o = fin.tile([128, D], BF16, name="o")
            nc.vector.tensor_scalar(o[:], pre[:], mv[:, 0:1], rstd[:, 0:1],
                                    op0=ALU.subtract, op1=ALU.mult)
            if gamma_out_b is not None:
                nc.vector.tensor_mul(o[:], o[:], gamma_out_b[:])
                nc.vector.tensor_add(o[:], o[:], beta_out_b[:])
            nc.scalar.dma_start(out=out_ext.ap()[m * 128:(m + 1) * 128, :], in_=o[:])


def _triv_flags(inputs):
    def ones(x):
        return bool(np.all(np.asarray(x) == 1.0))

    def zeros(x):
        return bool(np.all(np.asarray(x) == 0.0))

    return {
        "input_norm": ones(inputs["input_norm_gamma"]) and zeros(inputs["input_norm_beta"]),
        "projector_norm": ones(inputs["projector_norm_gamma"]) and zeros(inputs["projector_norm_beta"]),
        "output_norm": ones(inputs["output_norm_gamma"]) and zeros(inputs["output_norm_beta"]),
        "aggregator_bias": zeros(inputs["aggregator_bias"]),
        "projector_dense1_bias": zeros(inputs["projector_dense1_bias"]),
        "projector_dense2_bias": zeros(inputs["projector_dense2_bias"]),
        "broadcast_bias": zeros(inputs["broadcast_bias"]),
        "gate_bias": zeros(inputs["gate_bias"]),
    }


_GRAPH_CACHE = {}

BF16_INPUTS = ("hidden_states", "aggregator_weight", "projector_dense1_weight",
               "projector_dense2_weight", "broadcast_weight", "gate_weight")


def prep_in_maps(inputs):
    """Build per-core input maps: core r gets batch r of hidden_states plus
    its H-slice of W1/W2 and a one-hot batch selector; other weights are
    replicated.  Big tensors are host-cast to bf16."""
    import ml_dtypes
    hs = np.ascontiguousarray(
        np.asarray(inputs["hidden_states"], dtype=np.float32).astype(ml_dtypes.bfloat16))
    assert hs.shape == (B, L, D)
    com = {}
    for n in WEIGHT_NAMES:
        a = np.asarray(inputs[n], dtype=np.float32)
        if n in BF16_INPUTS:
            a = a.astype(ml_dtypes.bfloat16)
        com[n] = np.ascontiguousarray(a)
    in_maps = []
    for r in range(B):
        m = dict(com)
        m["hidden_states"] = np.ascontiguousarray(hs[r])
        m["projector_dense1_weight"] = np.ascontiguousarray(
            com["projector_dense1_weight"][:, r * HSL:(r + 1) * HSL])
        m["projector_dense2_weight"] = np.ascontiguousarray(
            com["projector_dense2_weight"][r * HSL:(r + 1) * HSL, :])
        m["projector_dense1_bias"] = np.ascontiguousarray(
            com["projector_dense1_bias"][r * HSL:(r + 1) * HSL])
        sel = np.zeros((B, 1), dtype=np.float32)
        sel[r, 0] = 1.0
        m["bsel"] = sel
        in_maps.append(m)
    return in_maps


def kernel(**inputs):
    triv = _triv_flags(inputs)
    key = tuple(sorted(triv.items()))
    if key not in _GRAPH_CACHE:
        _GRAPH_CACHE[key] = build_graph(triv)
    nc = _GRAPH_CACHE[key]
    in_maps = prep_in_maps(inputs)
    res = run_bass_kernel_spmd(nc, in_maps, core_ids=list(range(B)))
    out = np.stack([np.asarray(res.results[b]["out"]).astype(np.float32)
                    for b in range(B)], axis=0)
    return out
